# Initial kernel scaffold
#
"""Distributed causal single-head attention for 8 Trainium2 NeuronCores.

Problem: x [B=4, T=4096, E=1024] f32; Wq/Wk/Wv [E, H=64] f32.
out[b] = softmax(causal(q k^T / sqrt(H))) v,  q/k/v = x[b] @ W.

Sharding: core = (batch b = core//2, parity o = core%2). Each core computes
the output rows of the interleaved 512-row chunks {2J+o : J=0..3} of batch b.
The host ships x[b]^T (bf16, tile-blocked for contiguous DMA) with columns
permuted to [own-chunks | partner-chunks] so all 8 cores run one identical
(SPMD) instruction stream; parity enters only through data (a scalar mask
input).

Within a core:
  - q/k/v projections as bf16 matmuls (qk-pack [Wq|Wk], kv-pack [Wk|Wv]),
  - v^T tiles transposed to natural layout on the PE (identity matmul) —
    much cheaper than 32 serialized DMA-xbar transposes (~2us each),
  - scoresT[k,q] via row-packed K=64 matmul pairs writing both halves of a
    double-bank [128,1024] PSUM tile, so exp runs as ONE ScalarE activation
    per k-tile over two q-chunks (the ACT engine is the attention-phase
    bottleneck; halving its instruction count matters),
  - causal masks multiplied into the exp output on DVE,
  - PV as bf16 matmuls with an extra ones-row in v^T producing softmax
    denominators for free,
  - normalization via f32r reciprocal + K=1 broadcast matmul.
"""

import sys

sys.path.insert(0, "/opt/trn_rl_repo")

import numpy as np

import concourse.bass as bass  # noqa: F401
import concourse.tile as tile
from concourse import bacc, mybir
from concourse.bass_utils import run_bass_kernel_spmd

F32 = mybir.dt.float32
F32R = mybir.dt.float32r
BF16 = mybir.dt.bfloat16

B, T, E, H = 4, 4096, 1024, 64
N_CORES = 8
SCALE = float(H) ** -0.5


def build_attention(Eb=E, Tg=T, CH=512, n_loop=1, n_cores=N_CORES, unroll=1):
    """Per-core inputs:
    xb [2*NCH, 128, NE*CH] bf16  (xb[b,p,e*CH+c] = x_local[b*CH+c, e*128+p]),
    w  [NE, 128, 192] bf16       ([Wq | Wk | Wv] row-tiled),
    pm [128, 1] f32, ones [1, H] f32r  ->  outT [H, Tl] bf16.
    """
    Tl = Tg // 2
    TPC = CH // 128          # k-tiles per chunk
    NCH = Tl // CH           # local q-chunks per core
    NTl = Tl // 128          # own k-tiles
    NTg = 2 * NTl            # all k-tiles (own + partner)
    NE = Eb // 128           # contraction tiles
    NB = 2 * NCH             # column blocks (own + partner)
    VW = 128                 # padded v row width

    nc = bacc.Bacc("TRN2", target_bir_lowering=False, debug=False,
                   num_devices=n_cores)
    xb_ext = nc.dram_tensor("xb", [NB, 128, NE * CH], BF16, kind="ExternalInput")
    w_ext = nc.dram_tensor("w", [NE, 128, 192], BF16, kind="ExternalInput")
    pm_ext = nc.dram_tensor("pm", [128, 1], F32, kind="ExternalInput")
    ones_ext = nc.dram_tensor("ones", [1, H], F32R, kind="ExternalInput")
    outT_ext = nc.dram_tensor("outT", [H, Tl], BF16, kind="ExternalOutput")

    with tile.TileContext(nc) as tc:
        with (
            tc.tile_pool(name="const", bufs=1) as cpool,
            tc.tile_pool(name="big", bufs=1) as bigpool,
            tc.tile_pool(name="xs", bufs=4) as xpool,
            tc.tile_pool(name="work", bufs=4) as wpool,
            tc.tile_pool(name="psW", bufs=2, space="PSUM") as psW,
            tc.tile_pool(name="psP", bufs=2, space="PSUM") as psP,
            tc.tile_pool(name="psO", bufs=2, space="PSUM") as psO,
        ):
            # ---- constants ----
            w = cpool.tile([128, NE, 192], BF16, tag="w")
            tri = cpool.tile([128, TPC, CH], BF16, tag="tri")
            pm = cpool.tile([128, 1], F32, tag="pm")
            ones64 = cpool.tile([1, H], F32R, tag="ones64")
            id80 = cpool.tile([80, 80], BF16, tag="id80")
            for e in range(NE):
                nc.gpsimd.dma_start(out=w[:, e, :], in_=w_ext.ap()[e])
            nc.gpsimd.dma_start(out=pm[:, :], in_=pm_ext.ap())
            nc.gpsimd.dma_start(out=ones64[:, :], in_=ones_ext.ap())
            # tri[i][p, f] = 1.0 if 128*i + p <= f else 0.0
            for i in range(TPC):
                nc.gpsimd.memset(tri[:, i, :], 0.0)
                nc.gpsimd.affine_select(
                    out=tri[:, i, :], in_=tri[:, i, :],
                    compare_op=mybir.AluOpType.is_gt, fill=1.0,
                    base=128 * i, pattern=[[-1, CH]], channel_multiplier=1)
            # id80[p, f] = 1.0 iff p == f  (keep 1.0 where p-f==0, else 0)
            nc.gpsimd.memset(id80[:, :], 1.0)
            nc.gpsimd.affine_select(
                out=id80[:, :], in_=id80[:, :],
                compare_op=mybir.AluOpType.is_equal, fill=0.0,
                base=0, pattern=[[-1, 80]], channel_multiplier=1)

            def body(_iv=0, unroll=1):
                qdup = bigpool.tile([128, Tl], BF16, tag="qdup")
                kdup = bigpool.tile([128, Tg], BF16, tag="kdup")
                vT_sb = bigpool.tile([80, Tg], BF16, tag="vT_sb")
                nc.vector.memset(vT_sb[64:80, :], 1.0)
                v_nat = bigpool.tile([128, NTg, VW], BF16, tag="v_nat")

                # ---- projections ----
                def load_block(b):
                    xt = xpool.tile([128, NE, CH], BF16, tag="xT")
                    # alternate the big x loads between the HWDGE (sync) and
                    # SWDGE (gpsimd) DMA paths so they proceed in parallel
                    eng = nc.gpsimd if b in (3, 5, 7) else nc.sync
                    eng.dma_start(
                        out=xt[:, :, :],
                        in_=xb_ext.ap()[b].rearrange("p (e c) -> p e c", e=NE))
                    return xt

                def transpose_block(b):
                    # v^T [64(+16 ones), 128] -> v_nat [128, 80] on the PE.
                    # trp rides the psP ring (idle slots between proj tiles)
                    # to stay inside the 8-bank PSUM budget.
                    for i in range(TPC):
                        t = b * TPC + i
                        trp = psP.tile([128, 80], BF16, tag="proj")
                        nc.tensor.transpose(
                            trp[:], vT_sb[0:80, t * 128:(t + 1) * 128],
                            id80[:])
                        nc.vector.tensor_copy(v_nat[:, t, 0:80], trp[:])

                def proj_block(b):
                    own = b < NCH
                    xt = load_block(b)
                    cols = slice(b * CH, (b + 1) * CH)
                    ps = psP.tile([128, CH], F32, tag="proj")
                    woff = 0 if own else 64  # [Wq|Wk] or [Wk|Wv]
                    for e in range(NE):
                        nc.tensor.matmul(ps[:], w[:, e, woff:woff + 128],
                                         xt[:, e, :],
                                         start=(e == 0), stop=(e == NE - 1))
                    if own:
                        nc.vector.tensor_copy(qdup[0:64, cols], ps[0:64, :])
                        nc.vector.tensor_copy(qdup[64:128, cols], ps[0:64, :])
                        nc.vector.tensor_copy(kdup[0:64, cols], ps[64:128, :])
                        nc.vector.tensor_copy(kdup[64:128, cols], ps[64:128, :])
                        vps = psP.tile([128, CH], F32, tag="proj")
                        for e in range(NE):
                            nc.tensor.matmul(vps[0:64, :], w[:, e, 128:192],
                                             xt[:, e, :],
                                             start=(e == 0), stop=(e == NE - 1))
                        nc.vector.tensor_copy(vT_sb[0:64, cols], vps[0:64, :])
                    else:
                        nc.vector.tensor_copy(kdup[0:64, cols], ps[0:64, :])
                        nc.vector.tensor_copy(kdup[64:128, cols], ps[0:64, :])
                        nc.vector.tensor_copy(vT_sb[0:64, cols], ps[64:128, :])
                    transpose_block(b)

                # ---- attention: chunk pairs (2P, 2P+1) share one exp ----
                def apply_mask(ex_half, mask):
                    kind, i = mask
                    if kind == "tri":
                        nc.vector.tensor_mul(ex_half, ex_half, tri[:, i, :])
                    else:
                        nc.vector.tensor_scalar_mul(ex_half, ex_half,
                                                    pm[:, :])

                attn_state = {}

                def attn_open(P):
                    Jlo, Jhi = 2 * P, 2 * P + 1
                    # (tile, mask_lo) pairs; mask applies to the LO half
                    # (q-chunk Jlo); the HI half (q-chunk Jhi) sees the tile
                    # as fully causal-visible.
                    wide = []
                    for c in range(Jlo):
                        for i in range(TPC):
                            wide.append((TPC * c + i, None))
                    for c in range(Jlo):
                        for i in range(TPC):
                            wide.append((NTl + TPC * c + i, None))
                    for i in range(TPC):
                        wide.append((TPC * Jlo + i, ("tri", i)))
                    for i in range(TPC):
                        wide.append((NTl + TPC * Jlo + i, ("pm", 0)))
                    # hi-only diagonal tiles, processed as (own, partner)
                    # pairs sharing one [128,1024] exp
                    hip = [((TPC * Jhi + i, ("tri", i)),
                            (NTl + TPC * Jhi + i, ("pm", 0)))
                           for i in range(TPC)]
                    o_lo = psO.tile([H + 1, CH], F32, tag="o", name="o_lo")
                    o_hi = psO.tile([H + 1, CH], F32, tag="o", name="o_hi")
                    attn_state[P] = {"wide": wide, "hip": hip, "next_w": 0,
                                     "o_lo": o_lo, "o_hi": o_hi}

                def attn_wide(P, upto=None):
                    st = attn_state[P]
                    Jlo, Jhi = 2 * P, 2 * P + 1
                    qlo = slice(Jlo * CH, (Jlo + 1) * CH)
                    qhi = slice(Jhi * CH, (Jhi + 1) * CH)
                    wide, o_lo, o_hi = st["wide"], st["o_lo"], st["o_hi"]
                    n_w = len(wide)
                    stop_at = n_w if upto is None else upto
                    for wi in range(st["next_w"], stop_at):
                        t, mask = wide[wi]
                        c0 = slice(t * 128, t * 128 + 128)
                        psw = psW.tile([128, 2 * CH], F32, tag="s")
                        nc.tensor.matmul(psw[:, 0:CH], kdup[0:64, c0],
                                         qdup[0:64, qlo],
                                         start=True, stop=True)
                        nc.tensor.matmul(psw[:, CH:2 * CH], kdup[64:128, c0],
                                         qdup[64:128, qhi],
                                         start=True, stop=True)
                        ex = wpool.tile([128, 2 * CH], BF16, tag="ex")
                        nc.scalar.activation(ex[:], psw[:],
                                             mybir.ActivationFunctionType.Exp,
                                             scale=SCALE)
                        if mask is not None:
                            apply_mask(ex[:, 0:CH], mask)
                        nc.tensor.matmul(o_lo[:], v_nat[:, t, 0:H + 1],
                                         ex[:, 0:CH],
                                         start=(wi == 0), stop=(wi == n_w - 1))
                        nc.tensor.matmul(o_hi[:], v_nat[:, t, 0:H + 1],
                                         ex[:, CH:2 * CH],
                                         start=(wi == 0), stop=False)
                    st["next_w"] = stop_at

                def attn_close(P):
                    attn_wide(P)
                    st = attn_state[P]
                    Jlo, Jhi = 2 * P, 2 * P + 1
                    qlo = slice(Jlo * CH, (Jlo + 1) * CH)
                    qhi = slice(Jhi * CH, (Jhi + 1) * CH)
                    hip, o_lo, o_hi = st["hip"], st["o_lo"], st["o_hi"]
                    n_h = len(hip)
                    for hi_i, ((ta, ma), (tb, mb)) in enumerate(hip):
                        ca = slice(ta * 128, ta * 128 + 128)
                        cb = slice(tb * 128, tb * 128 + 128)
                        psw = psW.tile([128, 2 * CH], F32, tag="s")
                        nc.tensor.matmul(psw[:, 0:CH], kdup[0:64, ca],
                                         qdup[0:64, qhi],
                                         start=True, stop=True)
                        nc.tensor.matmul(psw[:, CH:2 * CH], kdup[64:128, cb],
                                         qdup[64:128, qhi],
                                         start=True, stop=True)
                        ex = wpool.tile([128, 2 * CH], BF16, tag="ex")
                        nc.scalar.activation(ex[:], psw[:],
                                             mybir.ActivationFunctionType.Exp,
                                             scale=SCALE)
                        apply_mask(ex[:, 0:CH], ma)
                        apply_mask(ex[:, CH:2 * CH], mb)
                        nc.tensor.matmul(o_hi[:], v_nat[:, ta, 0:H + 1],
                                         ex[:, 0:CH],
                                         start=False, stop=False)
                        nc.tensor.matmul(o_hi[:], v_nat[:, tb, 0:H + 1],
                                         ex[:, CH:2 * CH],
                                         start=False,
                                         stop=(hi_i == n_h - 1))

                    # normalize and store both chunks
                    for o_ps, qb in ((o_lo, qlo), (o_hi, qhi)):
                        recip = wpool.tile([1, CH], F32R, tag="recip")
                        with nc.allow_low_precision(reason="f32r recip"):
                            nc.vector.reciprocal(recip[:], o_ps[H:H + 1, :])
                        rb_ps = psP.tile([H, CH], F32, tag="proj")
                        nc.tensor.matmul(rb_ps[:], ones64[:], recip[:],
                                         start=True, stop=True)
                        o_sb0 = wpool.tile([H, CH], F32, tag="osb0")
                        nc.vector.tensor_copy(o_sb0[:], o_ps[0:H, :])
                        o_sb = wpool.tile([H, CH], BF16, tag="osb")
                        nc.vector.tensor_mul(o_sb[:], o_sb0[:], rb_ps[:])
                        nc.sync.dma_start(out=outT_ext.ap()[:, qb], in_=o_sb[:])

                # ---- emission: all projections, then attention pairs.
                # NB: interleaved/pipelined emission orders and partial-width
                # diagonal matmuls were both tried and REGRESSED 25-35us
                # (the Tile scheduler handles this simple phase structure
                # best); keep it sequential.
                for b in range(NB):
                    proj_block(b)
                for P in range(NCH // 2):
                    attn_open(P)
                    attn_close(P)

            if n_loop == 1:
                for _ in range(unroll):
                    body()
            else:
                with tc.For_i(0, n_loop, 1) as iv:
                    for _ in range(unroll):
                        body(iv)

    nc.compile()
    return nc


# ---------------- host-side shard / unshard ----------------

def make_in_maps(x, Wq, Wk, Wv, Tg=T, CH=512):
    import ml_dtypes
    Tl = Tg // 2
    NCH = Tl // CH
    NB = 2 * NCH
    NE = np.asarray(Wq).shape[0] // 128
    x = np.asarray(x)
    w_all = np.concatenate([np.asarray(Wq), np.asarray(Wk), np.asarray(Wv)],
                           axis=1).astype(ml_dtypes.bfloat16)     # [E, 192]
    w_tiled = np.ascontiguousarray(w_all.reshape(NE, 128, 192))

    in_maps = []
    for core in range(N_CORES):
        b, o = core // 2, core % 2
        own = [2 * J + o for J in range(NCH)]
        par = [2 * J + (1 - o) for J in range(NCH)]
        xl = np.concatenate([x[b, g * CH:(g + 1) * CH, :] for g in own + par],
                            axis=0)                               # [Tg, E]
        # xb[blk, p, e*CH + c] = xl[blk*CH + c, e*128 + p]
        xb = xl.reshape(NB, CH, NE, 128).transpose(0, 3, 2, 1)
        xb = np.ascontiguousarray(
            xb.reshape(NB, 128, NE * CH).astype(ml_dtypes.bfloat16))
        pmv = np.full((128, 1), 1.0 if o == 1 else 0.0, np.float32)
        in_maps.append({"xb": xb, "w": w_tiled, "pm": pmv,
                        "ones": np.ones((1, H), np.float32)})
    return in_maps


def unshard_out(results, Tg=T, CH=512):
    Tl = Tg // 2
    NCH = Tl // CH
    out = np.zeros((B, Tg, H), np.float32)
    for core in range(N_CORES):
        b, o = core // 2, core % 2
        outT = np.asarray(results[core]["outT"]).astype(np.float32)
        for J in range(NCH):
            g = 2 * J + o
            out[b, g * CH:(g + 1) * CH, :] = outT[:, J * CH:(J + 1) * CH].T
    return out


_cached_nc = None


def kernel(x, Wq, Wk, Wv):
    global _cached_nc
    if _cached_nc is None:
        _cached_nc = build_attention()
    in_maps = make_in_maps(x, Wq, Wk, Wv)
    res = run_bass_kernel_spmd(_cached_nc, in_maps, core_ids=list(range(N_CORES)))
    return unshard_out(res.results)



# revision 9
# speedup vs baseline: 1.0715x; 1.0715x over previous
"""Distributed causal single-head attention for 8 Trainium2 NeuronCores.

Problem: x [B=4, T=4096, E=1024] f32; Wq/Wk/Wv [E, H=64] f32.
out[b] = softmax(causal(q k^T / sqrt(H))) v,  q/k/v = x[b] @ W.

Sharding: core = (batch b = core//2, parity o = core%2). Each core computes
the output rows of the interleaved 512-row chunks {2J+o : J=0..3} of batch b.
The host ships x[b]^T (bf16, tile-blocked for contiguous DMA) with columns
permuted to [own-chunks | partner-chunks] so all 8 cores run one identical
(SPMD) instruction stream; parity enters only through data (a scalar mask
input).

Within a core:
  - q/k/v projections as bf16 matmuls (qk-pack [Wq|Wk], kv-pack [Wk|Wv]),
  - v^T tiles transposed to natural layout on the PE (identity matmul),
  - scoresT[k,q] via row-packed K=64 matmul pairs writing both halves of a
    double-bank [128,1024] PSUM tile, so exp runs as ONE ScalarE activation
    per k-tile over two q-chunks,
  - causal masks multiplied into the exp output on DVE/Pool,
  - PV as bf16 matmuls with an extra ones-row in v^T producing softmax
    denominators for free,
  - normalization via f32r reciprocal + K=1 broadcast matmul.

v2 (software pipelining): the projection blocks are interleaved with the
attention pairs so the ScalarE exp stream (the attention-phase bottleneck,
~1.04us per [128,1024] tile) starts as soon as the first two chunks are
projected, and the remaining projection blocks fill PE slack inside the
attention phases. Loop-invariant setup (big-buffer allocation, the
ones-rows memset of v^T, weight/const DMAs on the cheap HWDGE trigger
path) is hoisted out of the body. SBUF->SBUF duplicate copies ride the
Pool engine (no PSUM port, so PSUM evacuations stay on DVE); the two hip
masks of each diagonal pair run on DVE and Pool in parallel.

emission="rot" additionally rotates the pipeline across loop iterations:
blocks 0,1 of iteration i+1 are projected inside iteration i's ACT-bound
tail (P1 hip/norm), so the exp stream never starves at body boundaries.
The n_loop=1 build emits the same total work in pipe order.
"""

import sys

sys.path.insert(0, "/opt/trn_rl_repo")

import numpy as np

import concourse.bass as bass  # noqa: F401
import concourse.tile as tile
from concourse import bacc, mybir
from concourse.bass_utils import run_bass_kernel_spmd

F32 = mybir.dt.float32
F32R = mybir.dt.float32r
BF16 = mybir.dt.bfloat16

B, T, E, H = 4, 4096, 1024, 64
N_CORES = 8
SCALE = float(H) ** -0.5


def build_attention(Eb=E, Tg=T, CH=512, n_loop=1, n_cores=N_CORES, unroll=1,
                    emission="rot"):
    """Per-core inputs:
    xb [2*NCH, 128, NE*CH] bf16  (xb[b,p,e*CH+c] = x_local[b*CH+c, e*128+p]),
    w  [NE, 128, 192] bf16       ([Wq | Wk | Wv] row-tiled),
    pm [128, 1] f32, ones [1, H] f32r  ->  outT [H, Tl] bf16.
    """
    Tl = Tg // 2
    TPC = CH // 128          # k-tiles per chunk
    NCH = Tl // CH           # local q-chunks per core
    NTl = Tl // 128          # own k-tiles
    NTg = 2 * NTl            # all k-tiles (own + partner)
    NE = Eb // 128           # contraction tiles
    NB = 2 * NCH             # column blocks (own + partner)
    VW = 128                 # padded v row width

    nc = bacc.Bacc("TRN2", target_bir_lowering=False, debug=False,
                   num_devices=n_cores)
    xb_ext = nc.dram_tensor("xb", [NB, 128, NE * CH], BF16, kind="ExternalInput")
    w_ext = nc.dram_tensor("w", [NE, 128, 192], BF16, kind="ExternalInput")
    pm_ext = nc.dram_tensor("pm", [128, 1], F32, kind="ExternalInput")
    ones_ext = nc.dram_tensor("ones", [1, H], F32R, kind="ExternalInput")
    outT_ext = nc.dram_tensor("outT", [H, Tl], BF16, kind="ExternalOutput")

    with tile.TileContext(nc) as tc:
        with (
            tc.tile_pool(name="const", bufs=1) as cpool,
            tc.tile_pool(name="big", bufs=1) as bigpool,
            tc.tile_pool(name="xs", bufs=4) as xpool,
            tc.tile_pool(name="work", bufs=4) as wpool,
            tc.tile_pool(name="psW", bufs=2, space="PSUM") as psW,
            tc.tile_pool(name="psP", bufs=2, space="PSUM") as psP,
            tc.tile_pool(name="psO", bufs=2, space="PSUM") as psO,
        ):
            # ---- constants ----
            w = cpool.tile([128, NE, 192], BF16, tag="w")
            tri = cpool.tile([128, TPC, CH], BF16, tag="tri")
            pm = cpool.tile([128, 1], F32, tag="pm")
            ones64 = cpool.tile([1, H], F32R, tag="ones64")
            id80 = cpool.tile([80, 80], BF16, tag="id80")
            # one descriptor on the HWDGE path (SP trigger is cheap; Pool
            # triggers cost ~1us each and would starve the early pipeline)
            nc.sync.dma_start(
                out=w[:, :, :],
                in_=w_ext.ap().rearrange("e p c -> p e c"))
            nc.sync.dma_start(out=pm[:, :], in_=pm_ext.ap())
            nc.sync.dma_start(out=ones64[:, :], in_=ones_ext.ap())
            # tri[i][p, f] = 1.0 if 128*i + p <= f else 0.0
            for i in range(TPC):
                nc.gpsimd.memset(tri[:, i, :], 0.0)
                nc.gpsimd.affine_select(
                    out=tri[:, i, :], in_=tri[:, i, :],
                    compare_op=mybir.AluOpType.is_gt, fill=1.0,
                    base=128 * i, pattern=[[-1, CH]], channel_multiplier=1)
            # id80[p, f] = 1.0 iff p == f  (keep 1.0 where p-f==0, else 0)
            nc.gpsimd.memset(id80[:, :], 1.0)
            nc.gpsimd.affine_select(
                out=id80[:, :], in_=id80[:, :],
                compare_op=mybir.AluOpType.is_equal, fill=0.0,
                base=0, pattern=[[-1, 80]], channel_multiplier=1)

            # ---- loop-invariant big buffers (tag-reused across bodies) ----
            qdup = bigpool.tile([128, Tl], BF16, tag="qdup")
            kdup = bigpool.tile([128, Tg], BF16, tag="kdup")
            vT_sb = bigpool.tile([80, Tg], BF16, tag="vT_sb")
            v_nat = bigpool.tile([128, NTg, VW], BF16, tag="v_nat")
            nc.gpsimd.memset(vT_sb[64:80, :], 1.0)  # ones rows: written once

            # ---- projections ----
            def load_block(b):
                xt = xpool.tile([128, NE, CH], BF16, tag="xT")
                # alternate the big x loads between the HWDGE (sync) and
                # SWDGE (gpsimd) DMA paths so they proceed in parallel
                eng = nc.gpsimd if b in (1, 3, 5, 7) else nc.sync
                eng.dma_start(
                    out=xt[:, :, :],
                    in_=xb_ext.ap()[b].rearrange("p (e c) -> p e c", e=NE))
                return xt

            def transpose_block(b):
                # v^T [64(+16 ones), 128] -> v_nat [128, 80] on the PE.
                # trp rides the psP ring (idle slots between proj tiles)
                # to stay inside the 8-bank PSUM budget.
                for i in range(TPC):
                    t = b * TPC + i
                    trp = psP.tile([128, 80], BF16, tag="proj")
                    nc.tensor.transpose(
                        trp[:], vT_sb[0:80, t * 128:(t + 1) * 128],
                        id80[:])
                    nc.vector.tensor_copy(v_nat[:, t, 0:80], trp[:])

            def proj_block(b, dup_eng=None):
                # dup_eng: engine for the SBUF->SBUF duplicate copies
                # (Pool by default; DVE for the lead-in blocks where the
                # dup gates the first exp and DVE is otherwise idle)
                dup = dup_eng or nc.gpsimd
                own = b < NCH
                xt = load_block(b)
                cols = slice(b * CH, (b + 1) * CH)
                ps = psP.tile([128, CH], F32, tag="proj")
                woff = 0 if own else 64  # [Wq|Wk] or [Wk|Wv]
                for e in range(NE):
                    nc.tensor.matmul(ps[:], w[:, e, woff:woff + 128],
                                     xt[:, e, :],
                                     start=(e == 0), stop=(e == NE - 1))
                if own:
                    # PSUM evacuations on DVE; dups on dup_eng
                    nc.vector.tensor_copy(qdup[0:64, cols], ps[0:64, :])
                    dup.tensor_copy(qdup[64:128, cols], qdup[0:64, cols])
                    nc.vector.tensor_copy(kdup[0:64, cols], ps[64:128, :])
                    dup.tensor_copy(kdup[64:128, cols], kdup[0:64, cols])
                    vps = psP.tile([128, CH], F32, tag="proj")
                    for e in range(NE):
                        nc.tensor.matmul(vps[0:64, :], w[:, e, 128:192],
                                         xt[:, e, :],
                                         start=(e == 0), stop=(e == NE - 1))
                    nc.vector.tensor_copy(vT_sb[0:64, cols], vps[0:64, :])
                else:
                    nc.vector.tensor_copy(kdup[0:64, cols], ps[0:64, :])
                    dup.tensor_copy(kdup[64:128, cols], kdup[0:64, cols])
                    nc.vector.tensor_copy(vT_sb[0:64, cols], ps[64:128, :])
                transpose_block(b)

            # ---- attention: chunk pairs (2P, 2P+1) share one exp ----
            def apply_mask(ex_half, mask, eng=None):
                eng = eng or nc.vector
                kind, i = mask
                if kind == "tri":
                    eng.tensor_mul(ex_half, ex_half, tri[:, i, :])
                else:
                    eng.tensor_scalar_mul(ex_half, ex_half, pm[:, :])

            attn_state = {}

            def attn_open(P):
                Jlo, Jhi = 2 * P, 2 * P + 1
                # (tile, mask_lo) pairs; mask applies to the LO half
                # (q-chunk Jlo); the HI half (q-chunk Jhi) sees the tile
                # as fully causal-visible.
                wide = []
                for c in range(Jlo):
                    for i in range(TPC):
                        wide.append((TPC * c + i, None))
                for c in range(Jlo):
                    for i in range(TPC):
                        wide.append((NTl + TPC * c + i, None))
                for i in range(TPC):
                    wide.append((TPC * Jlo + i, ("tri", i)))
                for i in range(TPC):
                    wide.append((NTl + TPC * Jlo + i, ("pm", 0)))
                # hi-only diagonal tiles, processed as (own, partner)
                # pairs sharing one [128,1024] exp
                hip = [((TPC * Jhi + i, ("tri", i)),
                        (NTl + TPC * Jhi + i, ("pm", 0)))
                       for i in range(TPC)]
                o_lo = psO.tile([H + 1, CH], F32, tag="o", name="o_lo")
                o_hi = psO.tile([H + 1, CH], F32, tag="o", name="o_hi")
                attn_state[P] = {"wide": wide, "hip": hip, "next_w": 0,
                                 "o_lo": o_lo, "o_hi": o_hi}

            def attn_wide(P, upto=None):
                st = attn_state[P]
                Jlo, Jhi = 2 * P, 2 * P + 1
                qlo = slice(Jlo * CH, (Jlo + 1) * CH)
                qhi = slice(Jhi * CH, (Jhi + 1) * CH)
                wide, o_lo, o_hi = st["wide"], st["o_lo"], st["o_hi"]
                n_w = len(wide)
                stop_at = n_w if upto is None else upto
                for wi in range(st["next_w"], stop_at):
                    t, mask = wide[wi]
                    c0 = slice(t * 128, t * 128 + 128)
                    psw = psW.tile([128, 2 * CH], F32, tag="s")
                    nc.tensor.matmul(psw[:, 0:CH], kdup[0:64, c0],
                                     qdup[0:64, qlo],
                                     start=True, stop=True)
                    nc.tensor.matmul(psw[:, CH:2 * CH], kdup[64:128, c0],
                                     qdup[64:128, qhi],
                                     start=True, stop=True)
                    ex = wpool.tile([128, 2 * CH], BF16, tag="ex")
                    nc.scalar.activation(ex[:], psw[:],
                                         mybir.ActivationFunctionType.Exp,
                                         scale=SCALE)
                    if mask is not None:
                        apply_mask(ex[:, 0:CH], mask)
                    nc.tensor.matmul(o_lo[:], v_nat[:, t, 0:H + 1],
                                     ex[:, 0:CH],
                                     start=(wi == 0), stop=(wi == n_w - 1))
                    nc.tensor.matmul(o_hi[:], v_nat[:, t, 0:H + 1],
                                     ex[:, CH:2 * CH],
                                     start=(wi == 0), stop=False)
                st["next_w"] = stop_at

            def attn_hip(P):
                attn_wide(P)
                st = attn_state[P]
                Jlo, Jhi = 2 * P, 2 * P + 1
                qhi = slice(Jhi * CH, (Jhi + 1) * CH)
                hip, o_hi = st["hip"], st["o_hi"]
                n_h = len(hip)
                for hi_i, ((ta, ma), (tb, mb)) in enumerate(hip):
                    ca = slice(ta * 128, ta * 128 + 128)
                    cb = slice(tb * 128, tb * 128 + 128)
                    psw = psW.tile([128, 2 * CH], F32, tag="s")
                    nc.tensor.matmul(psw[:, 0:CH], kdup[0:64, ca],
                                     qdup[0:64, qhi],
                                     start=True, stop=True)
                    nc.tensor.matmul(psw[:, CH:2 * CH], kdup[64:128, cb],
                                     qdup[64:128, qhi],
                                     start=True, stop=True)
                    ex = wpool.tile([128, 2 * CH], BF16, tag="ex")
                    nc.scalar.activation(ex[:], psw[:],
                                         mybir.ActivationFunctionType.Exp,
                                         scale=SCALE)
                    # the two halves' masks run on DVE and Pool in parallel
                    apply_mask(ex[:, 0:CH], ma, eng=nc.vector)
                    apply_mask(ex[:, CH:2 * CH], mb, eng=nc.gpsimd)
                    nc.tensor.matmul(o_hi[:], v_nat[:, ta, 0:H + 1],
                                     ex[:, 0:CH],
                                     start=False, stop=False)
                    nc.tensor.matmul(o_hi[:], v_nat[:, tb, 0:H + 1],
                                     ex[:, CH:2 * CH],
                                     start=False,
                                     stop=(hi_i == n_h - 1))

            def attn_norm(P):
                st = attn_state[P]
                Jlo, Jhi = 2 * P, 2 * P + 1
                qlo = slice(Jlo * CH, (Jlo + 1) * CH)
                qhi = slice(Jhi * CH, (Jhi + 1) * CH)
                o_lo, o_hi = st["o_lo"], st["o_hi"]
                # normalize and store both chunks
                for o_ps, qb in ((o_lo, qlo), (o_hi, qhi)):
                    recip = wpool.tile([1, CH], F32R, tag="recip")
                    with nc.allow_low_precision(reason="f32r recip"):
                        nc.vector.reciprocal(recip[:], o_ps[H:H + 1, :])
                    rb_ps = psP.tile([H, CH], F32, tag="proj")
                    nc.tensor.matmul(rb_ps[:], ones64[:], recip[:],
                                     start=True, stop=True)
                    o_sb0 = wpool.tile([H, CH], F32, tag="osb0")
                    nc.vector.tensor_copy(o_sb0[:], o_ps[0:H, :])
                    o_sb = wpool.tile([H, CH], BF16, tag="osb")
                    nc.vector.tensor_mul(o_sb[:], o_sb0[:], rb_ps[:])
                    nc.sync.dma_start(out=outT_ext.ap()[:, qb], in_=o_sb[:])

            # ---- emission ----
            assert NCH == 4

            def body_main(tail):
                # assumes blocks 0,1 already projected (preamble or the
                # previous body's tail)
                attn_open(0)
                attn_wide(0, upto=TPC)        # own diag (blocks 0,1)
                proj_block(NCH + 0)
                attn_wide(0, upto=2 * TPC)    # partner diag (block 4)
                proj_block(NCH + 1)
                proj_block(2)
                attn_hip(0)                   # hip needs blocks 1, 5
                proj_block(3)
                attn_norm(0)
                attn_open(1)
                attn_wide(1, upto=2 * TPC)    # k chunks 0,1 (own)
                proj_block(NCH + 2)
                attn_wide(1, upto=4 * TPC)    # k chunks 0,1 (partner)
                proj_block(NCH + 3)
                if tail:
                    # next iteration's lead-in inside the ACT-bound tail
                    proj_block(0)
                    attn_hip(1)               # diag (2,6) + hip (3,7)
                    proj_block(1)
                    attn_norm(1)
                else:
                    attn_hip(1)
                    attn_norm(1)

            def body_base():
                for b in range(NB):
                    proj_block(b)
                for P in range(NCH // 2):
                    attn_open(P)
                    attn_hip(P)
                    attn_norm(P)

            if emission == "base":
                if n_loop == 1:
                    for _ in range(unroll):
                        body_base()
                else:
                    with tc.For_i(0, n_loop, 1) as iv:
                        for _ in range(unroll):
                            body_base()
            else:
                rot = emission == "rot"
                proj_block(0, dup_eng=nc.vector)   # lead-in (preamble)
                proj_block(1, dup_eng=nc.vector)
                if n_loop == 1:
                    for u in range(unroll):
                        last = u == unroll - 1
                        body_main(tail=rot and not last)
                        if not rot and not last:
                            proj_block(0)
                            proj_block(1)
                else:
                    with tc.For_i(0, n_loop, 1) as iv:
                        for u in range(unroll):
                            body_main(tail=rot)
                            if not rot:
                                proj_block(0)
                                proj_block(1)

    nc.compile()
    return nc


# ---------------- host-side shard / unshard ----------------

def make_in_maps(x, Wq, Wk, Wv, Tg=T, CH=512):
    import ml_dtypes
    Tl = Tg // 2
    NCH = Tl // CH
    NB = 2 * NCH
    NE = np.asarray(Wq).shape[0] // 128
    x = np.asarray(x)
    w_all = np.concatenate([np.asarray(Wq), np.asarray(Wk), np.asarray(Wv)],
                           axis=1).astype(ml_dtypes.bfloat16)     # [E, 192]
    w_tiled = np.ascontiguousarray(w_all.reshape(NE, 128, 192))

    in_maps = []
    for core in range(N_CORES):
        b, o = core // 2, core % 2
        own = [2 * J + o for J in range(NCH)]
        par = [2 * J + (1 - o) for J in range(NCH)]
        xl = np.concatenate([x[b, g * CH:(g + 1) * CH, :] for g in own + par],
                            axis=0)                               # [Tg, E]
        # xb[blk, p, e*CH + c] = xl[blk*CH + c, e*128 + p]
        xb = xl.reshape(NB, CH, NE, 128).transpose(0, 3, 2, 1)
        xb = np.ascontiguousarray(
            xb.reshape(NB, 128, NE * CH).astype(ml_dtypes.bfloat16))
        pmv = np.full((128, 1), 1.0 if o == 1 else 0.0, np.float32)
        in_maps.append({"xb": xb, "w": w_tiled, "pm": pmv,
                        "ones": np.ones((1, H), np.float32)})
    return in_maps


def unshard_out(results, Tg=T, CH=512):
    Tl = Tg // 2
    NCH = Tl // CH
    out = np.zeros((B, Tg, H), np.float32)
    for core in range(N_CORES):
        b, o = core // 2, core % 2
        outT = np.asarray(results[core]["outT"]).astype(np.float32)
        for J in range(NCH):
            g = 2 * J + o
            out[b, g * CH:(g + 1) * CH, :] = outT[:, J * CH:(J + 1) * CH].T
    return out


_cached_nc = None


def kernel(x, Wq, Wk, Wv):
    global _cached_nc
    if _cached_nc is None:
        _cached_nc = build_attention()
    in_maps = make_in_maps(x, Wq, Wk, Wv)
    res = run_bass_kernel_spmd(_cached_nc, in_maps, core_ids=list(range(N_CORES)))
    return unshard_out(res.results)
